# revision 1
# baseline (speedup 1.0000x reference)
"""BiATT kernel for 8 Trainium2 NeuronCores.

The reference module's bilinear-attention branch is dead code: the
"attention" weights are softmax(axis=1) over [N, 1] tensors, which is
exactly 1.0 for every row.  Hence

    cf_final = atoms_vector @ (Wcc[0:D] + Wcc[D:2D] + Wcc[2D:3D] + Wcc[3D:4D]) + bcc
    pf_final = amino_vector @ (Wcp[0:D] + Wcp[D:2D] + Wcp[2D:3D] + Wcp[3D:4D]) + bcp

bit-for-bit up to fp32 rounding, i.e. two [6144,512] @ [512,512] matmuls
with host-folded weights.

Default scheme "bf16s": stream-split sharding (cores 0-3 compute cf rows,
cores 4-7 pf rows, 1536 rows per core) with a SINGLE bf16 matmul term and
bf16 outputs, converted to f32 on the host.  Measured end-to-end error is
~3.5e-3 against the fp32 reference (the harness gate is 2e-2); per-core
traffic is 2.05 MB in + 1.57 MB out and 48 matmuls (805 MFLOP) -- one
third of the PE work and a quarter of the DMA bytes of the bf16x2 path.
See _build_bf16s for the pipeline schedule.  BIATT_MM selects the legacy
schemes (raw = hand-scheduled bf16x2, bf16x2 = Tile bf16x2, f32, f32r);
BIATT_NWARM tunes the warm-up burst.

The bias is added on the host during the gather (a rank-1 epilogue on the
full output).
"""

import os

import ml_dtypes
import numpy as np

import concourse.bacc as bacc
import concourse.bass as bass
import concourse.mybir as mybir
import concourse.tile as tile
from concourse.bass_utils import run_bass_kernel_spmd

N_CORES = 8
D = 512          # feature dim
N_ROWS = 6144    # rows of atoms_vector / amino_vector
SHARD = N_ROWS // N_CORES   # 768 rows per core
P = 128          # SBUF partitions
KC = D // P      # 4 contraction chunks
NRB = SHARD // P  # 6 row blocks per shard

_F32 = mybir.dt.float32
_BF16 = mybir.dt.bfloat16
_PROGRAM_CACHE = {}

_LAST_EXEC_NS = None


def _new_bass():
    return bacc.Bacc(
        "TRN2",
        target_bir_lowering=False,
        debug=False,
        num_devices=N_CORES,
    )


def _build_bf16x2():
    """Split-bf16 path: per stream (cc / cp) the activation comes as hi/lo
    bf16 halves and the folded weight as hi/lo bf16 halves.  Input tensors
    are partition-major K-chunked ([128, nk, len]) so each is one large
    contiguous DMA.  psum[rb] accumulates 12 matmuls: k0..3 of xh@wh,
    xl@wh, xh@wl.

    Perf structure: inputs are two-chunk halves loaded in consumption order
    on the Sync HWDGE ring (output DMAs ride the Activation ring so the two
    dispatch streams never serialize against each other); a burst of
    throwaway matmuls on scratch tiles keeps the PE busy during the DMA
    lead so the HAM clock gate is released (2.4 GHz) when the real matmul
    stream starts."""
    nc = _new_bass()

    # names: {tensor}{piece}; each tensor comes as 2 two-chunk halves.
    d = {}
    layout = {}
    for t, ln, npiece, nk in (
        ("xh", SHARD, 2, 2), ("wcch", D, 2, 2),
        ("xl", SHARD, 2, 2), ("wccl", D, 2, 2),
        ("yh", SHARD, 2, 2), ("wcph", D, 2, 2),
        ("yl", SHARD, 2, 2), ("wcpl", D, 2, 2),
    ):
        layout[t] = (ln, npiece, nk)
        for h in range(npiece):
            d[f"{t}{h}"] = nc.dram_tensor(
                f"{t}{h}", [P, nk, ln], _BF16, kind="ExternalInput"
            ).ap()

    cf = nc.dram_tensor("cf", [NRB, P, D], _F32, kind="ExternalOutput").ap()
    pf = nc.dram_tensor("pf", [NRB, P, D], _F32, kind="ExternalOutput").ap()

    with tile.TileContext(nc) as tc:
        with (
            tc.tile_pool(name="ins", bufs=1) as ins,
            tc.tile_pool(name="warm", bufs=1) as warm,
            tc.tile_pool(name="psum", bufs=7, space=bass.MemorySpace.PSUM) as psum,
            tc.tile_pool(name="wpsum", bufs=1, space=bass.MemorySpace.PSUM) as wpsum,
            tc.tile_pool(name="outs", bufs=8) as outs,
        ):
            # PE warm-up: ~4us of dependency-free matmuls on scratch data,
            # issued while the input DMAs stream in.  Keeps the HAM activity
            # window busy so the real matmuls run at 2.4 GHz from the start.
            wsrc = warm.tile([P, 2 * P], _BF16, tag="wsrc")
            nc.gpsimd.memset(wsrc[:], 0.0)
            wps = wpsum.tile([P, P], _F32, tag="wps")
            for i in range(40):
                nc.tensor.matmul(
                    wps[:], wsrc[:, 0:P], wsrc[:, P:2 * P],
                    start=(i == 0), stop=(i == 39),
                )

            # Load order == consumption order (cf stream first).
            s = {}
            def load(engine, name):
                ln, npiece, nk = layout[name[:-1]]
                t = ins.tile([P, nk, ln], _BF16, tag=name)
                engine.dma_start(t[:], d[name][:])
                s[name] = t

            for name in ("wcch0", "xh0", "wcch1", "xh1",
                         "xl0", "xl1", "wccl0", "wccl1",
                         "wcph0", "yh0", "wcph1", "yh1",
                         "yl0", "yl1", "wcpl0", "wcpl1"):
                load(nc.sync, name)

            def piece(t, k):
                ln, npiece, nk = layout[t]
                return s[f"{t}{k // nk}"][:, k % nk, :]

            for a, w, out_d in (("x", "wcc", cf), ("y", "wcp", pf)):
                for rb in range(NRB):
                    ps = psum.tile([P, D], _F32, tag="ps")
                    idx = 0
                    for ah, wh2 in ((f"{a}h", f"{w}h"), (f"{a}l", f"{w}h"),
                                    (f"{a}h", f"{w}l")):
                        for k in range(KC):
                            nc.tensor.matmul(
                                ps[:],
                                piece(ah, k)[:, rb * P:(rb + 1) * P],
                                piece(wh2, k),
                                start=(idx == 0),
                                stop=(idx == 3 * KC - 1),
                            )
                            idx += 1
                    ot = outs.tile([P, D], _F32, tag="ot")
                    nc.vector.tensor_copy(ot[:], ps[:])
                    nc.scalar.dma_start(out_d[rb], ot[:])

    nc.compile()
    return nc


_IN_ORDER = ("wcch0", "xh0", "wcch1", "xh1", "xl0", "xl1", "wccl0", "wccl1",
             "wcph0", "yh0", "wcph1", "yh1", "yl0", "yl1", "wcpl0", "wcpl1")

# ---------------------------------------------------------------------------
# Single-term bf16 scheme ("bf16s"): one stream per core (cores 0-3 compute
# cf rows, 4-7 pf rows; 1536 rows each), single bf16 matmul term (measured
# end-to-end error ~3.4e-3 vs the 2e-2 gate), bf16 outputs converted to f32
# on the host.  Per-core traffic: 2.05 MB in + 1.57 MB out vs the bf16x2
# path's 8.4 MB; PE work 48 matmuls (805 MFLOP) vs 144.
# ---------------------------------------------------------------------------

SHARD2 = N_ROWS // 4      # 1536 rows per core (4 cores per stream)
NRB2 = SHARD2 // P        # 12 row blocks
XH = SHARD2 // 2          # 768 columns per xT half (row-blocks 0-5 / 6-11)


# Output DMA plan: (blocks, queue) per DMA.  Early big pieces go on the
# Activation queue; the late pieces spread across queues so their
# HWDGE/SWDGE dispatch chains overlap.  "pool" DMAs use the SWDGE path,
# which does not occupy the shared HWDGE generator at all.
OUT_PLAN = ((3, "act"), (3, "act"), (2, "act"), (2, "act"), (1, "sync"),
            (1, "sync"))
OUT_SPLIT = tuple(n for n, _ in OUT_PLAN)


def _build_bf16s(nwarm=30, out_plan=OUT_PLAN):
    """Hand-scheduled raw pipeline, one [1536,512]@[512,512] bf16 matmul.

    DMA dispatch is the scarce resource: each HWDGE dispatch occupies the
    shared generator ~630ns, the first DMA's SEQ+HWDGE+DGE lead is ~2us,
    and every gate pays a 900ns completion-semaphore propagation.  The 8
    input pieces pack each weight K-chunk with the activation columns its
    gate unlocks, sized so each k-gate opens exactly when the PE reaches
    it at the DMA-engine cumulative-bytes limit: p0a=[wk0|rb0,rb1] first
    on the sync queue; p0b (k0 rb2-5) and xb23 ride the Pool/SWDGE path
    (no shared-HWDGE occupancy; xb23's dispatch waits for the p1 gate so
    its transfer cannot displace earlier pieces in the DMA-engine FIFO);
    p1/p2 = [wk|xa rb0-5]; p3a=[wk3|rb0] and p3b (rb1-5) split the k3
    gate the same way; xb01 carries the b-half k0/k1 chunks.

    The warm-up matmuls run on uninitialized SBUF from t~0 (their PSUM
    bank is reset by a later start=True) to hold the PE p-state ramp, and
    the ~5 instructions issued after the first (SEQ-blocking) gate run at
    the mid p-state, so they are burned on mov-32 throwaways -- every
    real matmul then runs at 2.4GHz, gap-free.

    Groups 0-5 (banks 0-5): k0,k1 k-outer, then per-rb (k2,k3); groups
    6-11 (banks 6,7,0-3; reuse gated on first-half copies): k0 k-outer,
    then per-rb (k1,k2,k3) so stops stagger 639ns, matching the 658ns
    DVE copy (GPSIMD cannot read PSUM).  Copies land in per-DMA SBUF
    slots; outputs go partition-major [128,12,512] (host re-transposes)
    via 3/3/2/2-block Activation-queue DMAs plus two single-block
    sync-queue DMAs for the tail, every DMA carrying a completion inc
    (neuronx-cc requires one)."""
    from contextlib import ExitStack

    nc = _new_bass()

    d = {}
    d["p0a"] = nc.dram_tensor("p0a", [P, D + 2 * P], _BF16, kind="ExternalInput").ap()
    d["p0b"] = nc.dram_tensor("p0b", [P, 4 * P], _BF16, kind="ExternalInput").ap()
    for k in range(1, KC - 1):
        d[f"p{k}"] = nc.dram_tensor(
            f"p{k}", [P, D + XH], _BF16, kind="ExternalInput").ap()
    d["p3a"] = nc.dram_tensor("p3a", [P, D + P], _BF16, kind="ExternalInput").ap()
    d["p3b"] = nc.dram_tensor("p3b", [P, 5 * P], _BF16, kind="ExternalInput").ap()
    d["xb01"] = nc.dram_tensor("xb01", [P, 2, XH], _BF16, kind="ExternalInput").ap()
    d["xb23"] = nc.dram_tensor("xb23", [P, 2, XH], _BF16, kind="ExternalInput").ap()
    out_d = nc.dram_tensor("out", [P, NRB2, D], _BF16, kind="ExternalOutput").ap()

    out_split = tuple(n for n, _ in out_plan)
    DMA_ORDER = ("p0a", "p0b", "p1", "p2", "p3a", "p3b", "xb01", "xb23")
    SHAPES = {"p0a": [P, D + 2 * P], "p0b": [P, 4 * P],
              "p1": [P, D + XH], "p2": [P, D + XH],
              "p3a": [P, D + P], "p3b": [P, 5 * P],
              "xb01": [P, 2, XH], "xb23": [P, 2, XH]}
    NJ = len(out_split)

    with ExitStack() as ctx:
        sb = {
            name: ctx.enter_context(
                nc.sbuf_tensor(f"sb_{name}", SHAPES[name], _BF16))
            for name in DMA_ORDER
        }
        # One private slot per output DMA -- no ping-pong waits anywhere.
        max_n = max(out_split)
        outsb = ctx.enter_context(
            nc.sbuf_tensor("outsb", [P, NJ, max_n * D], _BF16))
        warm = ctx.enter_context(nc.sbuf_tensor("warmsb", [P, 2 * P], _BF16))
        ps = [
            ctx.enter_context(nc.psum_tensor(f"psum{i}", [P, D], _F32))
            for i in range(8)
        ]
        s_mm = ctx.enter_context(nc.semaphore("s_mm"))
        s_cpv = ctx.enter_context(nc.semaphore("s_cpv"))  # DVE copies
        s_cpg = ctx.enter_context(nc.semaphore("s_cpg"))  # GPSIMD copies
        s_cpa = ctx.enter_context(nc.semaphore("s_cpa"))  # ACT copies
        s_od = ctx.enter_context(nc.semaphore("s_od"))    # out-DMA completions
        gates = {
            name: ctx.enter_context(nc.semaphore(f"s_{name}"))
            for name in DMA_ORDER
        }

        blk_dma = []
        for j, n in enumerate(out_split):
            for o in range(n):
                blk_dma.append((j, o))
        dma_first_blk = [sum(out_split[:j]) for j in range(NJ)]
        # GPSIMD cannot read PSUM (BIR verifier); all copies ride DVE,
        # whose 658ns per block matches the phase-b stop stagger (639ns).
        cp_eng = [0] * NRB2
        cp_sem_of = {0: s_cpv, 1: s_cpg, 2: s_cpa}

        def cp_counts(last_blk):
            return tuple(
                sum(1 for g in range(NRB2) if cp_eng[g] == e and g <= last_blk)
                for e in range(3)
            )

        def wchunk(k):
            if k == 0:
                return sb["p0a"][:, 0:D]
            if k == 3:
                return sb["p3a"][:, 0:D]
            return sb[f"p{k}"][:, 0:D]

        def xblk(half, k, rb):
            if half == "a":
                if k == 0:
                    t, r = ("p0a", rb) if rb < 2 else ("p0b", rb - 2)
                    off = D if t == "p0a" else 0
                    return sb[t][:, off + r * P:off + (r + 1) * P]
                if k == 3:
                    t, r = ("p3a", rb) if rb < 1 else ("p3b", rb - 1)
                    off = D if t == "p3a" else 0
                    return sb[t][:, off + r * P:off + (r + 1) * P]
                return sb[f"p{k}"][:, D + rb * P:D + (rb + 1) * P]
            return sb["xb01" if k < 2 else "xb23"][:, k % 2, rb * P:(rb + 1) * P]

        def copy_loop(eng_idx, engine, eng_ns, sem):
            for g in range(NRB2):
                if cp_eng[g] != eng_idx:
                    continue
                j, o = blk_dma[g]
                engine.wait_ge(s_mm, g + 1)
                eng_ns.tensor_copy(
                    outsb[:, j, o * D:(o + 1) * D], ps[g % 8][:]
                ).then_inc(sem, 1)

        with nc.Block() as block:

            def issue_out(handle, eng_ns, j):
                n = out_split[j]
                b0 = dma_first_blk[j]
                counts = cp_counts(b0 + n - 1)
                for e, cnt in enumerate(counts):
                    if cnt:
                        handle.wait_ge(cp_sem_of[e], cnt)
                eng_ns.dma_start(
                    out_d[:, b0:b0 + n, :], outsb[:, j, 0:n * D]
                ).then_inc(s_od, 16)

            def act_copy(scalar, g):
                j, o = blk_dma[g]
                scalar.wait_ge(s_mm, g + 1)
                nc.scalar.activation(
                    outsb[:, j, o * D:(o + 1) * D], ps[g % 8][:],
                    mybir.ActivationFunctionType.Copy,
                ).then_inc(s_cpa, 1)

            @block.sync
            def _(sync):
                for name in DMA_ORDER:
                    if name in ("p0b", "xb23"):
                        continue  # issued on the Pool/SWDGE queue
                    sync.dma_start(sb[name][:], d[name][:]).then_inc(
                        gates[name], 16
                    )
                for j, (n, q) in enumerate(out_plan):
                    if q == "sync":
                        issue_out(sync, nc.sync, j)

            @block.vector
            def _(vector):
                copy_loop(0, vector, nc.vector, s_cpv)
                for j, (n, q) in enumerate(out_plan):
                    if q == "vec":
                        issue_out(vector, nc.vector, j)

            @block.gpsimd
            def _(gpsimd):
                # p0b rides SWDGE (no shared-HWDGE occupancy) so the sync
                # chain's HWDGE slots all go to the k1..k3 gate pieces;
                # xb23 is consumed last and dispatches only after p1 lands
                # so its transfer does not displace the earlier gates.
                nc.gpsimd.dma_start(sb["p0b"][:], d["p0b"][:]).then_inc(
                    gates["p0b"], 16
                )
                gpsimd.wait_ge(gates["p1"], 16)
                nc.gpsimd.dma_start(sb["xb23"][:], d["xb23"][:]).then_inc(
                    gates["xb23"], 16
                )
                for j, (n, q) in enumerate(out_plan):
                    if q == "pool":
                        issue_out(gpsimd, nc.gpsimd, j)
                for j, (n, q) in enumerate(out_plan):
                    if q == "pool":
                        issue_out(gpsimd, nc.gpsimd, j)

            @block.tensor
            def _(tensor):
                for i in range(nwarm):
                    nc.tensor.matmul(
                        ps[7][:, 0:P], warm[:, 0:P], warm[:, P:2 * P],
                        start=(i == 0), stop=(i == nwarm - 1),
                    )
                waited = set()

                def gate(name):
                    if name not in waited:
                        waited.add(name)
                        tensor.wait_ge(gates[name], 16)

                def bank_wait(g):
                    if g >= 8:
                        e = cp_eng[g - 8]
                        tensor.wait_ge(cp_sem_of[e], cp_counts(g - 8)[e])

                def mm(half, k, rb, start, stop):
                    g = (0 if half == "a" else 6) + rb
                    if start:
                        bank_wait(g)
                    m = nc.tensor.matmul(
                        ps[g % 8][:], xblk(half, k, rb), wchunk(k),
                        start=start, stop=stop,
                    )
                    if stop:
                        m.then_inc(s_mm, 1)

                # Phase a: the first gate blocks the PE SEQ, and the ~5
                # instructions issued after any blocking wait run at the
                # mid p-state -- burn them on short throwaway matmuls so
                # every real matmul runs at full clock.
                gate("p0a")
                for i in range(5):
                    nc.tensor.matmul(
                        ps[7][:, 0:32], warm[:, 0:P], warm[:, P:P + 32],
                        start=(i == 0), stop=(i == 4),
                    )
                for rb in range(6):
                    if rb == 2:
                        gate("p0b")
                    mm("a", 0, rb, True, False)
                gate("p1")
                for rb in range(6):
                    mm("a", 1, rb, False, False)
                gate("p2")
                gate("p3a")
                for rb in range(6):
                    if rb == 1:
                        gate("p3b")
                    mm("a", 2, rb, False, False)
                    mm("a", 3, rb, False, True)
                # Phase b: k0 k-outer; per-rb (k1,k2,k3) so stops stagger
                # 639ns apart, matching the DVE copy throughput.
                gate("xb01")
                for rb in range(6):
                    mm("b", 0, rb, True, False)
                for rb in range(6):
                    mm("b", 1, rb, False, False)
                    if rb == 0:
                        gate("xb23")
                    mm("b", 2, rb, False, False)
                    mm("b", 3, rb, False, True)

            @block.scalar
            def _(scalar):
                # Dummy activation so the Copy act-table is loaded long
                # before the first real copy.
                nc.scalar.activation(
                    outsb[:, 0, 0:32], warm[:, 0:32],
                    mybir.ActivationFunctionType.Copy,
                )
                # Interleave ACT copies and this queue's out-DMAs in
                # dependency order: copy g keyed g, DMA j keyed by its last
                # block + 0.5, so every DMA follows the copies it waits on.
                items = []
                for j, (n, q) in enumerate(out_plan):
                    if q == "act":
                        items.append((dma_first_blk[j] + n - 0.5, "dma", j))
                for g in range(NRB2):
                    if cp_eng[g] == 2:
                        items.append((float(g), "cp", g))
                for _, kind, idx in sorted(items):
                    if kind == "dma":
                        issue_out(scalar, nc.scalar, idx)
                    else:
                        act_copy(scalar, idx)

        nc.compile()
    return nc


def _get_program(scheme):
    if scheme not in _PROGRAM_CACHE:
        if scheme == "bf16s":
            _PROGRAM_CACHE[scheme] = _build_bf16s(
                nwarm=int(os.environ.get("BIATT_NWARM", "24"))
            )
        elif scheme == "raw":
            _PROGRAM_CACHE[scheme] = _build_raw()
        elif scheme == "bf16x2":
            _PROGRAM_CACHE[scheme] = _build_bf16x2()
        else:
            _PROGRAM_CACHE[scheme] = _build_f32(
                mybir.dt.float32r if scheme == "f32r" else _F32
            )
    return _PROGRAM_CACHE[scheme]


def _chunk_pieces(mat_t, dtype, npiece):
    """[K=512, len] -> npiece contiguous [128, 4/npiece, len] partition-major
    K-chunk groups."""
    ln = mat_t.shape[1]
    c = np.ascontiguousarray(
        mat_t.reshape(KC, P, ln).transpose(1, 0, 2).astype(dtype)
    )  # [128, 4, len]
    per = KC // npiece
    return [np.ascontiguousarray(c[:, i * per:(i + 1) * per]) for i in range(npiece)]


def _chunk_halves(mat_t, dtype):
    return _chunk_pieces(mat_t, dtype, 2)


def _split_hi_lo(a):
    hi = a.astype(ml_dtypes.bfloat16)
    lo = (a - hi.astype(np.float32)).astype(ml_dtypes.bfloat16)
    return hi, lo


def kernel(**inputs):
    global _LAST_EXEC_NS

    atoms = np.ascontiguousarray(np.asarray(inputs["atoms_vector"], dtype=np.float32))
    amino = np.ascontiguousarray(np.asarray(inputs["amino_vector"], dtype=np.float32))
    Wcc = np.asarray(inputs["Wcc"], dtype=np.float32)
    Wcp = np.asarray(inputs["Wcp"], dtype=np.float32)
    bcc = np.asarray(inputs["bcc"], dtype=np.float32)
    bcp = np.asarray(inputs["bcp"], dtype=np.float32)

    # Fold the four weight blocks (concat([v]*4, 1) @ W == v @ sum-of-blocks).
    wcc_f = Wcc.reshape(4, D, D).sum(axis=0)
    wcp_f = Wcp.reshape(4, D, D).sum(axis=0)

    scheme = os.environ.get("BIATT_MM", "bf16s")
    nc = _get_program(scheme)

    in_maps = []
    if scheme == "bf16s":
        # Stream-split sharding: cores 0-3 compute cf rows (atoms @ wcc_f),
        # cores 4-7 pf rows (amino @ wcp_f); 1536 rows per core.
        w_bf = {
            "cc": wcc_f.astype(ml_dtypes.bfloat16),
            "cp": wcp_f.astype(ml_dtypes.bfloat16),
        }
        for c in range(N_CORES):
            stream = "cc" if c < 4 else "cp"
            base = atoms if c < 4 else amino
            sl = slice((c % 4) * SHARD2, (c % 4 + 1) * SHARD2)
            xt = base[sl].T.astype(ml_dtypes.bfloat16)  # [512, 1536]
            m = {}
            wb = w_bf[stream]
            xb = np.empty((P, 2, 2, XH), dtype=ml_dtypes.bfloat16)
            for k in range(KC):
                chunk = xt[k * P:(k + 1) * P]
                if k == 0:
                    p0a = np.empty((P, D + 2 * P), dtype=ml_dtypes.bfloat16)
                    p0a[:, :D] = wb[:P]
                    p0a[:, D:] = chunk[:, :2 * P]
                    m["p0a"] = p0a
                    m["p0b"] = np.ascontiguousarray(chunk[:, 2 * P:XH])
                elif k == 3:
                    p3a = np.empty((P, D + P), dtype=ml_dtypes.bfloat16)
                    p3a[:, :D] = wb[k * P:(k + 1) * P]
                    p3a[:, D:] = chunk[:, :P]
                    m["p3a"] = p3a
                    m["p3b"] = np.ascontiguousarray(chunk[:, P:XH])
                else:
                    pk = np.empty((P, D + XH), dtype=ml_dtypes.bfloat16)
                    pk[:, :D] = wb[k * P:(k + 1) * P]
                    pk[:, D:] = chunk[:, :XH]
                    m[f"p{k}"] = pk
                xb[:, k // 2, k % 2] = chunk[:, XH:]
            m["xb01"] = np.ascontiguousarray(xb[:, 0])
            m["xb23"] = np.ascontiguousarray(xb[:, 1])
            in_maps.append(m)
    elif scheme in ("bf16x2", "raw"):
        # raw: wcch/xh in four per-chunk pieces, the rest in two halves;
        # tile bf16x2: everything in two halves.
        n_first = 2
        wcch, wccl = _split_hi_lo(wcc_f)
        wcph, wcpl = _split_hi_lo(wcp_f)
        w_parts = {}
        for nm, arr, npiece in (("wcch", wcch, n_first), ("wccl", wccl, 2),
                                ("wcph", wcph, 2), ("wcpl", wcpl, 2)):
            for i, p in enumerate(_chunk_pieces(arr, ml_dtypes.bfloat16, npiece)):
                w_parts[f"{nm}{i}"] = p
        for c in range(N_CORES):
            sl = slice(c * SHARD, (c + 1) * SHARD)
            m = dict(w_parts)
            for nm, base in (("x", atoms), ("y", amino)):
                t = base[sl].T  # [512, 768]
                hi, lo = _split_hi_lo(t)
                nh = n_first if nm == "x" else 2
                for i, p in enumerate(_chunk_pieces(hi, ml_dtypes.bfloat16, nh)):
                    m[f"{nm}h{i}"] = p
                for i, p in enumerate(_chunk_pieces(lo, ml_dtypes.bfloat16, 2)):
                    m[f"{nm}l{i}"] = p
            in_maps.append(m)
    else:
        w_parts = {}
        for nm, arr in (("wcc", wcc_f), ("wcp", wcp_f)):
            w_parts[f"{nm}0"], w_parts[f"{nm}1"] = _chunk_halves(arr, np.float32)
        for c in range(N_CORES):
            sl = slice(c * SHARD, (c + 1) * SHARD)
            m = dict(w_parts)
            m["x0"], m["x1"] = _chunk_halves(atoms[sl].T, np.float32)
            m["y0"], m["y1"] = _chunk_halves(amino[sl].T, np.float32)
            in_maps.append(m)

    trace = bool(os.environ.get("BIATT_TRACE"))
    try:
        res = run_bass_kernel_spmd(nc, in_maps, list(range(N_CORES)), trace=trace)
    except Exception:
        # One retry: a transiently wedged NeuronCore surfaces as a runtime
        # error on an otherwise-valid program.
        res = run_bass_kernel_spmd(nc, in_maps, list(range(N_CORES)), trace=trace)
    _LAST_EXEC_NS = res.exec_time_ns

    if scheme == "bf16s":
        def _unpack(c):
            # Device layout [128, 12, 512] (partition-major) -> [1536, 512].
            o = res.results[c]["out"]
            return o.transpose(1, 0, 2).reshape(SHARD2, D).astype(np.float32)

        cf = np.concatenate([_unpack(c) for c in range(4)], axis=0)
        pf = np.concatenate([_unpack(c) for c in range(4, 8)], axis=0)
    else:
        cf = np.concatenate(
            [res.results[c]["cf"].reshape(SHARD, D) for c in range(N_CORES)],
            axis=0,
        )
        pf = np.concatenate(
            [res.results[c]["pf"].reshape(SHARD, D) for c in range(N_CORES)],
            axis=0,
        )
    cf += bcc  # rank-1 epilogue on the gathered output
    pf += bcp
    return cf, pf

